# revision 1
# baseline (speedup 1.0000x reference)
"""Trainium2 Bass kernel for GQA multi-head attention with RoPE (causal).

Problem: B=2, T=2048, H=2048, NH=32 q-heads, NKV=8 kv-heads, HD=64.
  q = x@wq.T, k = x@wk.T, v = x@wv.T; RoPE(q, k) interleaved-pair style;
  causal softmax(q k^T / 8) @ v; out @ wo.T.

Sharding: 4 q-heads + 1 kv-head per core (8 cores, tensor-parallel heads);
each core computes a full-shape partial of the output projection, host sums.

Device layout (per core) is feature-major ("transposed") throughout:
  xT [H, B*T] -> Q.T [256, T]/batch, K.T-dup [128, T]/batch, V tok-major
  S.T[k, q] tiles via row-packed pairs (2 heads share the 128-wide PE array,
  K=64 each), exp on ACT straight out of PSUM, AV + rowsum matmuls col-packed,
  normalization via partition-broadcast multiply, output proj back to yT.
"""

import sys
from contextlib import ExitStack

import numpy as np

sys.path.insert(0, "/opt/trn_rl_repo")

import concourse.bass as bass  # noqa: E402
import concourse.bacc as bacc  # noqa: E402
import concourse.tile as tile  # noqa: E402
from concourse import mybir  # noqa: E402

F32 = mybir.dt.float32
F32R = mybir.dt.float32r
BF16 = mybir.dt.bfloat16
EXP = mybir.ActivationFunctionType.Exp

B, T, H = 2, 2048, 2048
NH, NKV, HD = 32, 8, 64
NCORES = 8
QH = NH // NCORES          # 4 q heads per core
QCH = QH * HD              # 256 q channels per core
NPAIR = QH // 2            # 2 head pairs per core
HT = 128                   # contraction tile over H
QT = 512                   # query tile
KT = 128                   # key tile
NDIAG = QT // KT           # diagonal sub-block count
SCALE = 1.0 / 8.0          # 1/sqrt(HD)
BASE = 10000.0

# even<->odd partition swap within each 32-block (interleaved rotate_half)
SHUF_MASK = [x for i in range(16) for x in (2 * i + 1, 2 * i)]


def build_nc(Bv=B, Tv=T, mmdt=F32R):
    NTOK = Bv * Tv
    NHT = H // HT          # 16 contraction tiles
    NTOKT = Tv // QT       # token tiles per batch (proj uses QT-wide tiles too)
    NQT = Tv // QT         # q tiles per batch
    NKTILE = Tv // KT      # k tiles per batch

    nc = bacc.Bacc("TRN2", target_bir_lowering=False, debug=False)
    xT = nc.dram_tensor("xT", [H, NTOK], mmdt, kind="ExternalInput")
    wqT = nc.dram_tensor("wqT", [H, QCH], mmdt, kind="ExternalInput")
    wkvT = nc.dram_tensor("wkvT", [H, 2 * HD], mmdt, kind="ExternalInput")
    woT = nc.dram_tensor("woT", [QCH, H], mmdt, kind="ExternalInput")
    cosT = nc.dram_tensor("cosT", [128, Tv], F32, kind="ExternalInput")
    sinT = nc.dram_tensor("sinT", [128, Tv], F32, kind="ExternalInput")
    masks = nc.dram_tensor("masks", [NDIAG, 128, QT], mmdt, kind="ExternalInput")
    ident = nc.dram_tensor("ident", [64, 64], F32, kind="ExternalInput")
    onesd = nc.dram_tensor("onesd", [128, 1], mmdt, kind="ExternalInput")
    yT = nc.dram_tensor("yT", [H, NTOK], F32, kind="ExternalOutput")

    def mm(x):
        return x

    def bc_in(ap):
        return ap

    with tile.TileContext(nc) as tc, ExitStack() as ctx:
        const = ctx.enter_context(tc.tile_pool(name="const", bufs=1))
        perb = ctx.enter_context(tc.tile_pool(name="perb", bufs=1))
        work = ctx.enter_context(tc.tile_pool(name="work", bufs=2))
        dramp = ctx.enter_context(tc.tile_pool(name="dramp", bufs=4, space="DRAM"))

        # ---- constants ----
        wq_sb = const.tile([128, NHT, QCH], mmdt, tag="wq")
        nc.sync.dma_start(out=wq_sb, in_=bc_in(wqT.rearrange("(n p) m -> p n m", p=128)))
        wkv_sb = const.tile([128, NHT, 2 * HD], mmdt, tag="wkv")
        nc.sync.dma_start(out=wkv_sb, in_=bc_in(wkvT.rearrange("(n p) m -> p n m", p=128)))
        wo_sb = const.tile([128, 2, H], mmdt, tag="wo")
        nc.sync.dma_start(out=wo_sb, in_=bc_in(woT.rearrange("(c p) h -> p c h", p=128)))
        cos_sb = const.tile([128, Tv], F32, tag="cos")
        nc.sync.dma_start(out=cos_sb, in_=cosT[:, :])
        sin_sb = const.tile([128, Tv], F32, tag="sin")
        nc.sync.dma_start(out=sin_sb, in_=sinT[:, :])
        mask_sb = const.tile([128, NDIAG, QT], mmdt, tag="mask")
        nc.sync.dma_start(out=mask_sb, in_=masks.rearrange("n p q -> p n q"))
        id_sb = const.tile([64, 64], F32, tag="ident")
        nc.sync.dma_start(out=id_sb, in_=ident[:, :])
        ones_sb = const.tile([128, 1], mmdt, tag="ones")
        nc.sync.dma_start(out=ones_sb, in_=bc_in(onesd[:, :]))

        # ---- persistent per-batch tensors ----
        qt_sb = [
            perb.tile([128, 2, Tv], mmdt, tag=f"qt{b}", name=f"qt{b}")
            for b in range(Bv)
        ]
        kd_sb = [
            perb.tile([128, Tv], mmdt, tag=f"kd{b}", name=f"kd{b}") for b in range(Bv)
        ]
        v_sb = [
            perb.tile([128, NKTILE, HD + 1], mmdt, tag=f"v{b}", name=f"v{b}")
            for b in range(Bv)
        ]

        # ================= phase 1: projections + RoPE + V transpose ========
        with tc.tile_pool(name="projps", bufs=1, space="PSUM") as projps:
            for bi in range(Bv):
                vt_b = perb.tile([64, Tv], F32, tag="vt")
                for j in range(NTOKT):
                    g0 = bi * Tv + j * QT
                    t0 = j * QT
                    tsl = slice(t0, t0 + QT)
                    p3 = projps.tile([128, 3, QT], F32, tag="p3")
                    for kg in range(NHT // 4):
                        xt = work.tile([128, 4, QT], mmdt, tag="xt", bufs=3)
                        x0 = xT[kg * 512 : kg * 512 + 128, g0 : g0 + QT]
                        nc.sync.dma_start(
                            out=xt,
                            in_=bass.AP(
                                x0.tensor, x0.offset,
                                [[NTOK, 128], [128 * NTOK, 4], [1, QT]],
                            ),
                        )
                        for k4 in range(4):
                            k = kg * 4 + k4
                            st, sp = (k == 0), (k == NHT - 1)
                            nc.tensor.matmul(
                                p3[:, 0, :], mm(wq_sb[:, k, 0:128]),
                                mm(xt[:, k4, :]), start=st, stop=sp,
                            )
                            nc.tensor.matmul(
                                p3[:, 1, :], mm(wq_sb[:, k, 128:256]),
                                mm(xt[:, k4, :]), start=st, stop=sp,
                            )
                            nc.tensor.matmul(
                                p3[:, 2, :], mm(wkv_sb[:, k, :]),
                                mm(xt[:, k4, :]), start=st, stop=sp,
                            )
                    # -- evacuate Q channel tiles with RoPE --
                    for ct in range(2):
                        qraw = work.tile([128, QT], F32, tag="qraw", bufs=3)
                        nc.scalar.copy(qraw, p3[:, ct, :])
                        shuf = work.tile([128, QT], F32, tag="shuf", bufs=3)
                        nc.vector.stream_shuffle(shuf, qraw, SHUF_MASK)
                        dst = qt_sb[bi][:, ct, tsl]
                        nc.vector.tensor_mul(dst, qraw, cos_sb[:, tsl])
                        nc.gpsimd.tensor_mul(shuf, shuf, sin_sb[:, tsl])
                        nc.vector.tensor_add(dst, dst, shuf)
                    # -- K rows 0:64 with RoPE, duplicated into both halves --
                    kraw = work.tile([64, QT], F32, tag="kraw")
                    nc.scalar.copy(kraw, p3[0:64, 2, :])
                    kshuf = work.tile([64, QT], F32, tag="kshuf")
                    nc.vector.stream_shuffle(kshuf, kraw, SHUF_MASK)
                    ktmp = work.tile([64, QT], F32, tag="ktmp")
                    nc.vector.tensor_mul(ktmp, kraw, cos_sb[0:64, tsl])
                    nc.gpsimd.tensor_mul(kshuf, kshuf, sin_sb[0:64, tsl])
                    nc.vector.tensor_add(kd_sb[bi][0:64, tsl], ktmp, kshuf)
                    nc.vector.tensor_add(kd_sb[bi][64:128, tsl], ktmp, kshuf)
                    # -- V rows 64:128 -> staging (cross-partition DVE copy) --
                    nc.vector.tensor_copy(vt_b[0:64, tsl], p3[64:128, 2, :])
                # ones column for the fused rowsum trick
                nc.sync.dma_start(
                    out=v_sb[bi][:, :, HD : HD + 1],
                    in_=bc_in(
                        bass.AP(onesd, 0, [[1, 128], [0, NKTILE], [1, 1]])
                    ),
                )
                # -- V: feature-major -> token-major via PE transpose --
                for kt in range(NKTILE):
                    vtp = projps.tile([128, HD], F32, tag="vtp", bufs=2)
                    nc.tensor.transpose(
                        vtp, vt_b[0:64, kt * 128 : (kt + 1) * 128], id_sb
                    )
                    nc.vector.tensor_copy(v_sb[bi][:, kt, 0:HD], vtp)

        # ================= phase 2: attention + output projection ===========
        with tc.tile_pool(name="attnps", bufs=1, space="PSUM") as attnps:
            for bi in range(Bv):
                for qi in range(NQT):
                    q0 = qi * QT
                    n_k = min(q0 // KT + NDIAG, NKTILE)
                    ots = []
                    for pp in range(NPAIR):
                        ua = attnps.tile([HD + 1, QT], F32, tag="u", bufs=2)
                        ub = attnps.tile([HD + 1, QT], F32, tag="u", bufs=2)
                        for ki in range(n_k):
                            k0 = ki * KT
                            d = (k0 - q0) // KT  # >=0 on diagonal blocks
                            c0 = max(k0 - q0, 0)
                            st, sp = (ki == 0), (ki == n_k - 1)
                            s = attnps.tile([128, 2, QT], F32, tag="s", bufs=2)
                            nc.tensor.matmul(
                                s[:, 0, c0:QT],
                                mm(kd_sb[bi][0:64, k0 : k0 + KT]),
                                mm(qt_sb[bi][0:64, pp, q0 + c0 : q0 + QT]),
                                tile_position=(0, 0),
                                start=True, stop=True, skip_group_check=True,
                            )
                            nc.tensor.matmul(
                                s[:, 1, c0:QT],
                                mm(kd_sb[bi][64:128, k0 : k0 + KT]),
                                mm(qt_sb[bi][64:128, pp, q0 + c0 : q0 + QT]),
                                tile_position=(64, 0),
                                start=True, stop=True, skip_group_check=True,
                            )
                            e = work.tile([128, 2, QT], mmdt, tag="e", bufs=4)
                            nc.scalar.activation(
                                e[:, :, c0:QT], s[:, :, c0:QT], EXP, scale=SCALE
                            )
                            if d >= 0:
                                nc.vector.tensor_mul(
                                    e[:, 0, c0:QT], e[:, 0, c0:QT],
                                    mask_sb[:, d, c0:QT],
                                )
                                nc.vector.tensor_mul(
                                    e[:, 1, c0:QT], e[:, 1, c0:QT],
                                    mask_sb[:, d, c0:QT],
                                )
                            vb = v_sb[bi][:, ki, :]
                            nc.tensor.matmul(
                                ua[:, c0:QT], mm(vb), mm(e[:, 0, c0:QT]),
                                start=st, stop=sp, skip_group_check=True,
                            )
                            nc.tensor.matmul(
                                ub[:, c0:QT], mm(vb), mm(e[:, 1, c0:QT]),
                                start=st, stop=sp, skip_group_check=True,
                            )
                        # normalize: O.T = U / rowsum (broadcast along partitions)
                        rec = work.tile([HD + 1, QT], F32, tag="rec")
                        nc.vector.reciprocal(rec[HD : HD + 1, :], ua[HD : HD + 1, :])
                        rec2 = work.tile([HD + 1, QT], F32, tag="rec")
                        nc.vector.reciprocal(rec2[HD : HD + 1, :], ub[HD : HD + 1, :])
                        sc = dramp.tile([2, QT], F32, tag="sc")
                        nc.sync.dma_start(out=sc[0:1, :], in_=rec[HD : HD + 1, :])
                        nc.sync.dma_start(out=sc[1:2, :], in_=rec2[HD : HD + 1, :])
                        bc = work.tile([64, QT], F32, tag="bc")
                        bc2 = work.tile([64, QT], F32, tag="bc")
                        s0, s1 = sc[0:1, :], sc[1:2, :]
                        nc.sync.dma_start(
                            out=bc[0:64, :],
                            in_=bass.AP(s0.tensor, s0.offset, [[0, 64], [1, QT]]),
                        )
                        nc.sync.dma_start(
                            out=bc2[0:64, :],
                            in_=bass.AP(s1.tensor, s1.offset, [[0, 64], [1, QT]]),
                        )
                        ot = work.tile([128, QT], mmdt, tag="ot", bufs=4)
                        nc.vector.tensor_mul(ot[0:64, :], ua[0:HD, :], bc)
                        nc.vector.tensor_mul(ot[64:128, :], ub[0:HD, :], bc2)
                        ots.append(ot)
                    # -- output projection for this (batch, q-tile) --
                    for oi in range(NHT):
                        y = attnps.tile([128, QT], F32, tag="y", bufs=2)
                        osl = slice(oi * 128, (oi + 1) * 128)
                        nc.tensor.matmul(
                            y, mm(wo_sb[:, 0, osl]), mm(ots[0]),
                            start=True, stop=False,
                        )
                        nc.tensor.matmul(
                            y, mm(wo_sb[:, 1, osl]), mm(ots[1]),
                            start=False, stop=True,
                        )
                        ysb = work.tile([128, QT], F32, tag="ysb", bufs=4)
                        if oi % 2 == 0:
                            nc.scalar.copy(ysb, y)
                        else:
                            nc.vector.tensor_copy(ysb, y)
                        nc.sync.dma_start(
                            out=yT[osl, bi * Tv + q0 : bi * Tv + q0 + QT], in_=ysb
                        )
    nc.finalize()
    return nc


def host_inputs(x, wq, wk, wv, wo, Bv=B, Tv=T, mmdt=F32R):
    """Shard + pre-transpose inputs; returns list of 8 per-core input dicts."""
    if mmdt is BF16:
        import ml_dtypes

        cast = lambda a: np.ascontiguousarray(a).astype(ml_dtypes.bfloat16)
    else:
        cast = lambda a: np.ascontiguousarray(a, dtype=np.float32)
    NTOK = Bv * Tv
    xT = cast(x.reshape(NTOK, H).T)

    # RoPE tables matching reference: emb = concat([freqs, freqs]) over dim,
    # rotate_half interleaved; sign folded into sin rows.
    inv_freq = (1.0 / (BASE ** (np.arange(0, HD, 2, dtype=np.float32) / np.float32(HD)))).astype(np.float32)
    t = np.arange(Tv, dtype=np.float32)
    freqs = np.outer(t, inv_freq)                       # [T, 32]
    emb = np.concatenate([freqs, freqs], axis=-1)       # [T, 64]
    cos = np.cos(emb).astype(np.float32)                # [T, 64]
    sin = np.sin(emb).astype(np.float32)
    sgn = np.where(np.arange(HD) % 2 == 0, -1.0, 1.0).astype(np.float32)
    sinS = sin * sgn[None, :]                           # sign-folded
    cosT2 = np.ascontiguousarray(np.vstack([cos.T, cos.T]))   # [128, T]
    sinT2 = np.ascontiguousarray(np.vstack([sinS.T, sinS.T]))  # [128, T]

    ki = np.arange(KT)[:, None]
    qi = np.arange(QT)[None, :]
    masks = np.stack(
        [(qi >= d * KT + ki).astype(np.float32) for d in range(NDIAG)]
    )                                                    # [NDIAG, 128, QT]
    ident = np.eye(64, dtype=np.float32)
    onesd = cast(np.ones((128, 1), dtype=np.float32))
    masks = cast(masks)

    in_maps = []
    for c in range(NCORES):
        qs = slice(c * QCH, (c + 1) * QCH)
        ks = slice(c * HD, (c + 1) * HD)
        wqT = cast(wq[qs].T)                             # [H, 256]
        wkvT = cast(np.concatenate([wk[ks].T, wv[ks].T], axis=1))  # [H, 128]
        woT = cast(wo[:, qs].T)                          # [256, H]
        in_maps.append(
            dict(xT=xT, wqT=wqT, wkvT=wkvT, woT=woT, cosT=cosT2, sinT=sinT2,
                 masks=masks, ident=ident, onesd=onesd)
        )
    return in_maps


_CACHED = {}


MMDT = BF16


def kernel(x, wq, wk, wv, wo):
    from concourse.bass_utils import run_bass_kernel_spmd

    if "nc" not in _CACHED:
        _CACHED["nc"] = build_nc(mmdt=MMDT)
    nc = _CACHED["nc"]
    in_maps = host_inputs(x, wq, wk, wv, wo, mmdt=MMDT)
    res = run_bass_kernel_spmd(nc, in_maps, core_ids=list(range(NCORES)))
    y = np.zeros((H, B * T), dtype=np.float64)
    for c in range(NCORES):
        y += res.results[c]["yT"].astype(np.float64)
    return np.ascontiguousarray(y.T.astype(np.float32).reshape(B, T, H))



# revision 30
# speedup vs baseline: 1.6892x; 1.6892x over previous
"""Trainium2 Bass kernel for GQA multi-head attention with RoPE (causal).

Problem: B=2, T=2048, H=2048, NH=32 q-heads, NKV=8 kv-heads, HD=64.
  q = x@wq.T, k = x@wk.T, v = x@wv.T; RoPE(q, k) interleaved-pair style;
  causal softmax(q k^T / 8) @ v; out @ wo.T.

Sharding: 4 q-heads + 1 kv-head per core (8 cores, tensor-parallel heads);
each core computes a full-shape partial of the output projection, host sums
partials (the TP all-reduce) in float64 on the host.

v3 design notes (all per core, feature-major/"transposed" on device):
  - PE p-state: TRN2 ramps 1.2->2.4 GHz only after ~3us of gap-free PE
    execution; every stall resets the ramp. The kernel keeps the PE
    streaming: double-buffered projection PSUM, scores issued 2 blocks
    ahead of AV, causal mask as an additive PE matmul (identity x
    mask-bias into the scores PSUM), and output-projection matmuls
    interleaved one-unit-per-block into the NEXT q-tile's attention
    stream (a pending-work queue) so softmax normalization latency never
    stalls the PE.
  - RoPE: projections evacuate PSUM once via the scalar engine into a
    bf16 staging tile; all shuffle/mul/add run on bf16 SBUF operands
    (2x DVE mode), sin-muls on the otherwise-idle gpsimd engine.
  - softmax: rowsum rides as a 65th row of V (ones column via memset);
    1/rowsum via DVE reciprocal_approx_fast -> gpsimd partition_broadcast
    -> one DVE multiply per head. No DRAM roundtrips.
  - output: fp16 partials (half the DMA bytes of fp32; ~5e-4 rel noise).
"""

import sys
from contextlib import ExitStack

import numpy as np

sys.path.insert(0, "/opt/trn_rl_repo")

import concourse.bass as bass  # noqa: E402
import concourse.bacc as bacc  # noqa: E402
import concourse.tile as tile  # noqa: E402
from concourse import mybir  # noqa: E402

F32 = mybir.dt.float32
F16 = mybir.dt.float16
BF16 = mybir.dt.bfloat16
EXP = mybir.ActivationFunctionType.Exp

B, T, H = 2, 2048, 2048
NH, NKV, HD = 32, 8, 64
NCORES = 8
QH = NH // NCORES          # 4 q heads per core
QCH = QH * HD              # 256 q channels per core
NPAIR = QH // 2            # 2 head pairs per core
HT = 128                   # contraction tile over H
QT = 512                   # query tile
KT = 128                   # key tile
NDIAG = QT // KT           # diagonal sub-block count
SCALE = 1.0 / 8.0          # 1/sqrt(HD)
BASE = 10000.0
MASKVAL = -30000.0         # additive causal bias (exp((s+m)/8) == 0)

# even<->odd partition swap within each 32-block (interleaved rotate_half)
SHUF_MASK = [x for i in range(16) for x in (2 * i + 1, 2 * i)]

# causal mask scheme: True = additive bias via PE matmul into scores PSUM;
# False = multiplicative 0/1 mask on DVE after exp
MASK_ON_PE = False
DEBUG_TAPS = False


def build_nc(Bv=B, Tv=T, mmdt=BF16):
    NTOK = Bv * Tv
    NHT = H // HT          # 16 contraction tiles
    NTOKT = Tv // QT       # token tiles per batch
    NQT = Tv // QT         # q tiles per batch
    NKTILE = Tv // KT      # k tiles per batch

    nc = bacc.Bacc("TRN2", target_bir_lowering=False, debug=False)
    xT = nc.dram_tensor("xT", [H, NTOK], mmdt, kind="ExternalInput")
    wqT = nc.dram_tensor("wqT", [H, QCH], mmdt, kind="ExternalInput")
    wkvT = nc.dram_tensor("wkvT", [H, 2 * HD], mmdt, kind="ExternalInput")
    woT = nc.dram_tensor("woT", [QCH, H], mmdt, kind="ExternalInput")
    cosT = nc.dram_tensor("cosT", [128, Tv], mmdt, kind="ExternalInput")
    sinT = nc.dram_tensor("sinT", [128, Tv], mmdt, kind="ExternalInput")
    masks = nc.dram_tensor("masks", [NDIAG, 128, QT], mmdt, kind="ExternalInput")
    ident = nc.dram_tensor("ident", [128, 128], mmdt, kind="ExternalInput")
    yT = nc.dram_tensor("yT", [H, NTOK], F16, kind="ExternalOutput")
    if DEBUG_TAPS:
        dbg_qt = nc.dram_tensor("dbg_qt", [128, 2, Tv], F32, kind="ExternalOutput")
        dbg_kd = nc.dram_tensor("dbg_kd", [128, Tv], F32, kind="ExternalOutput")
        dbg_v = nc.dram_tensor("dbg_v", [128, NKTILE, HD + 1], F32, kind="ExternalOutput")
        dbg_u = nc.dram_tensor("dbg_u", [HD + 1, QT], F32, kind="ExternalOutput")
        dbg_rb = nc.dram_tensor("dbg_rb", [64, QT], F32, kind="ExternalOutput")
        dbg_ot = nc.dram_tensor("dbg_ot", [128, QT], F32, kind="ExternalOutput")

    with tile.TileContext(nc) as tc, ExitStack() as ctx:
        const = ctx.enter_context(tc.tile_pool(name="const", bufs=1))
        perb = ctx.enter_context(tc.tile_pool(name="perb", bufs=1))
        work = ctx.enter_context(tc.tile_pool(name="work", bufs=2))

        # ---- constants; wq/wkv DMA first (they gate the first matmul),
        # the rest are emitted lazily after the first x tile's DMA ----
        wq_sb = const.tile([128, NHT, QCH], mmdt, tag="wq")
        nc.sync.dma_start(out=wq_sb, in_=wqT.rearrange("(n p) m -> p n m", p=128))
        wkv_sb = const.tile([128, NHT, 2 * HD], mmdt, tag="wkv")
        nc.sync.dma_start(out=wkv_sb, in_=wkvT.rearrange("(n p) m -> p n m", p=128))
        cos_sb = const.tile([128, Tv], mmdt, tag="cos")
        sin_sb = const.tile([128, Tv], mmdt, tag="sin")
        id_sb = const.tile([128, 128], mmdt, tag="ident")
        mask_sb = const.tile([128, NDIAG, QT], mmdt, tag="mask")
        wo_sb = const.tile([128, 2, H], mmdt, tag="wo")

        def emit_late_consts():
            nc.sync.dma_start(out=cos_sb, in_=cosT[:, :])
            nc.sync.dma_start(out=sin_sb, in_=sinT[:, :])
            nc.sync.dma_start(out=id_sb, in_=ident[:, :])
            nc.sync.dma_start(out=mask_sb, in_=masks.rearrange("n p q -> p n q"))
            nc.sync.dma_start(out=wo_sb, in_=woT.rearrange("(c p) h -> p c h", p=128))

        # ---- persistent per-batch tensors ----
        qt_sb = [
            perb.tile([128, 2, Tv], mmdt, tag=f"qt{b}", name=f"qt{b}")
            for b in range(Bv)
        ]
        kd_sb = [
            perb.tile([128, Tv], mmdt, tag=f"kd{b}", name=f"kd{b}") for b in range(Bv)
        ]
        v_sb = [
            perb.tile([128, NKTILE, HD + 1], mmdt, tag=f"v{b}", name=f"v{b}")
            for b in range(Bv)
        ]

        # pending output-projection units; each is a closure(psum_pool)
        pending = []

        def drain(pool, n):
            for _ in range(min(n, len(pending))):
                pending.pop(0)(pool)

        def queue_oproj(bi, q0, ots):
            for oi in range(NHT):
                def unit(pool, bi=bi, q0=q0, ots=ots, oi=oi):
                    y = pool.tile([128, QT], F32, tag="y2", bufs=2)
                    osl = slice(oi * 128, (oi + 1) * 128)
                    nc.tensor.matmul(
                        y, wo_sb[:, 0, osl], ots[0], start=True, stop=False,
                    )
                    nc.tensor.matmul(
                        y, wo_sb[:, 1, osl], ots[1], start=False, stop=True,
                    )
                    # gpsimd cannot read PSUM; evacuation is DVE-only
                    ysb = work.tile([128, QT], F16, tag="ysb", bufs=4)
                    nc.vector.tensor_copy(ysb, y)
                    nc.sync.dma_start(
                        out=bass.AP(
                            yT, oi * 128 * NTOK + bi * Tv + q0,
                            [[NTOK, 128], [1, QT]],
                        ),
                        in_=ysb,
                    )
                pending.append(unit)

        # xt DMA emission with cross-batch prefetch
        xt_tiles = {}

        def emit_xt_dma(bi, j, kg):
            g0 = bi * Tv + j * QT
            xt = work.tile([128, 4, QT], mmdt, tag="xt", bufs=4)
            x0 = xT[kg * 512 : kg * 512 + 128, g0 : g0 + QT]
            nc.sync.dma_start(
                out=xt,
                in_=bass.AP(
                    x0.tensor, x0.offset,
                    [[NTOK, 128], [128 * NTOK, 4], [1, QT]],
                ),
            )
            if bi == 0 and j == 0 and kg == 0:
                emit_late_consts()
            return xt

        def emit_xt(bi, j, kg):
            t = xt_tiles.pop((bi, j, kg), None)
            return t if t is not None else emit_xt_dma(bi, j, kg)

        def prefetch_xt(bi, j):
            for kg in range(NHT // 4):
                xt_tiles[(bi, j, kg)] = emit_xt_dma(bi, j, kg)

        for bi in range(Bv):
            # ============ phase 1: projections + RoPE + V transpose =========
            with tc.tile_pool(name=f"projps{bi}", bufs=1, space="PSUM") as projps:
                nc.vector.memset(v_sb[bi][:, :, HD : HD + 1], 1.0)
                for j in range(NTOKT):
                    g0 = bi * Tv + j * QT
                    t0 = j * QT
                    tsl = slice(t0, t0 + QT)
                    p3 = projps.tile([128, 3, QT], F32, tag="p3", bufs=2)
                    for kg in range(NHT // 4):
                        xt = emit_xt(bi, j, kg)
                        for k4 in range(4):
                            k = kg * 4 + k4
                            st, sp = (k == 0), (k == NHT - 1)
                            nc.tensor.matmul(
                                p3[:, 0, :], wq_sb[:, k, 0:128],
                                xt[:, k4, :], start=st, stop=sp,
                            )
                            nc.tensor.matmul(
                                p3[:, 1, :], wq_sb[:, k, 128:256],
                                xt[:, k4, :], start=st, stop=sp,
                            )
                            nc.tensor.matmul(
                                p3[:, 2, :], wkv_sb[:, k, :],
                                xt[:, k4, :], start=st, stop=sp,
                            )
                    # -- single fp32->bf16 PSUM evacuation on the ACT engine --
                    praw = work.tile([128, 3, QT], mmdt, tag="praw", bufs=2)
                    nc.scalar.copy(praw, p3)
                    # -- Q RoPE: all-bf16 SBUF ops (2x DVE), sin-mul on pool --
                    qshuf = work.tile([128, 2, QT], mmdt, tag="qshuf", bufs=2)
                    nc.vector.stream_shuffle(qshuf, praw[:, 0:2, :], SHUF_MASK)
                    sin2 = bass.AP(
                        sin_sb.tensor, sin_sb.offset + t0,
                        [[Tv, 128], [0, 2], [1, QT]],
                    )
                    cos2 = bass.AP(
                        cos_sb.tensor, cos_sb.offset + t0,
                        [[Tv, 128], [0, 2], [1, QT]],
                    )
                    nc.vector.tensor_mul(qshuf, qshuf, sin2)
                    dst = qt_sb[bi][:, :, tsl]
                    nc.vector.tensor_mul(dst, praw[:, 0:2, :], cos2)
                    nc.vector.tensor_add(dst, dst, qshuf)
                    # -- K rows 0:64 with RoPE, duplicated into both halves --
                    kshuf = work.tile([64, QT], mmdt, tag="kshuf", bufs=2)
                    nc.vector.stream_shuffle(kshuf, praw[0:64, 2, :], SHUF_MASK)
                    nc.vector.tensor_mul(kshuf, kshuf, sin_sb[0:64, tsl])
                    ktmp = work.tile([64, QT], mmdt, tag="ktmp", bufs=2)
                    nc.vector.tensor_mul(ktmp, praw[0:64, 2, :], cos_sb[0:64, tsl])
                    nc.vector.tensor_add(kd_sb[bi][0:64, tsl], ktmp, kshuf)
                    nc.vector.tensor_add(kd_sb[bi][64:128, tsl], ktmp, kshuf)
                    # -- V rows 64:128 (no RoPE): PE transpose from praw --
                    for c in range(QT // 128):
                        kt = j * 4 + c
                        vtp = projps.tile([128, HD], mmdt, tag="vtp", bufs=2)
                        nc.tensor.transpose(
                            vtp, praw[64:128, 2, c * 128 : (c + 1) * 128],
                            id_sb[64:128, 64:128], tile_position=(64, 0),
                        )
                        nc.scalar.copy(v_sb[bi][:, kt, 0:HD], vtp)
                if DEBUG_TAPS and bi == 0:
                    qtf = work.tile([128, 2, Tv], F32, tag="dbgqt")
                    nc.vector.tensor_copy(qtf, qt_sb[0])
                    nc.sync.dma_start(out=dbg_qt[:, :, :], in_=qtf)
                    kdf = work.tile([128, Tv], F32, tag="dbgkd")
                    nc.vector.tensor_copy(kdf, kd_sb[0])
                    nc.sync.dma_start(out=dbg_kd[:, :], in_=kdf)
                    vf = work.tile([128, NKTILE, HD + 1], F32, tag="dbgv")
                    nc.vector.tensor_copy(vf, v_sb[0])
                    nc.sync.dma_start(out=dbg_v[:, :, :], in_=vf)
                if bi + 1 < Bv:
                    # warm the DMA pipe for the next batch's first tile
                    prefetch_xt(bi + 1, 0)

            # ============ phase 2: attention + interleaved o-proj ===========
            with tc.tile_pool(name=f"attnps{bi}", bufs=1, space="PSUM") as attnps:
                for qi in range(NQT):
                    q0 = qi * QT
                    n_k = min(q0 // KT + NDIAG, NKTILE)
                    ots = []
                    for pp in range(NPAIR):
                        ua = attnps.tile([HD + 1, QT], F32, tag="u", bufs=2)
                        ub = attnps.tile([HD + 1, QT], F32, tag="u", bufs=2)
                        s_tiles = {}

                        def emit_s(ki, pp=pp, q0=q0, s_tiles=s_tiles):
                            k0 = ki * KT
                            c0 = max(k0 - q0, 0)
                            diag = k0 >= q0
                            pe_mask = diag and MASK_ON_PE
                            s = attnps.tile([128, 2, QT], F32, tag="s", bufs=2)
                            nc.tensor.matmul(
                                s[:, 0, c0:QT],
                                kd_sb[bi][0:64, k0 : k0 + KT],
                                qt_sb[bi][0:64, pp, q0 + c0 : q0 + QT],
                                tile_position=(0, 0),
                                start=True, stop=not pe_mask, skip_group_check=True,
                            )
                            nc.tensor.matmul(
                                s[:, 1, c0:QT],
                                kd_sb[bi][64:128, k0 : k0 + KT],
                                qt_sb[bi][64:128, pp, q0 + c0 : q0 + QT],
                                tile_position=(64, 0),
                                start=True, stop=not pe_mask, skip_group_check=True,
                            )
                            if pe_mask:
                                d = (k0 - q0) // KT
                                nc.tensor.matmul(
                                    s[:, 0, c0:QT], id_sb,
                                    mask_sb[:, d, c0:QT],
                                    start=False, stop=True, skip_group_check=True,
                                )
                                nc.tensor.matmul(
                                    s[:, 1, c0:QT], id_sb,
                                    mask_sb[:, d, c0:QT],
                                    start=False, stop=True, skip_group_check=True,
                                )
                            s_tiles[ki] = (s, c0)

                        emit_s(0)
                        if n_k > 1:
                            emit_s(1)
                        for ki in range(n_k):
                            s, c0 = s_tiles.pop(ki)
                            e = work.tile([128, 2, QT], mmdt, tag="e", bufs=3)
                            nc.scalar.activation(
                                e[:, :, c0:QT], s[:, :, c0:QT], EXP, scale=SCALE
                            )
                            if ki * KT >= q0 and not MASK_ON_PE:
                                d = (ki * KT - q0) // KT
                                nc.vector.tensor_mul(
                                    e[:, :, c0:QT], e[:, :, c0:QT],
                                    bass.AP(
                                        mask_sb.tensor,
                                        mask_sb.offset + d * QT + c0,
                                        [[NDIAG * QT, 128], [0, 2], [1, QT - c0]],
                                    ),
                                )
                            drain(attnps, 2 if len(pending) > 16 else 1)
                            if ki + 2 < n_k:
                                emit_s(ki + 2)
                            vb = v_sb[bi][:, ki, :]
                            st, sp = (ki == 0), (ki == n_k - 1)
                            nc.tensor.matmul(
                                ua[:, c0:QT], vb, e[:, 0, c0:QT],
                                start=st, stop=sp, skip_group_check=True,
                            )
                            nc.tensor.matmul(
                                ub[:, c0:QT], vb, e[:, 1, c0:QT],
                                start=st, stop=sp, skip_group_check=True,
                            )
                        # normalize: O.T = U * bcast(1/rowsum)
                        # (recip_approx_fast mis-reads PSUM inputs on HW;
                        # stage the rowsum rows through SBUF first)
                        rina = work.tile([1, QT], F32, tag="rin", bufs=4)
                        nc.vector.tensor_copy(rina, ua[HD : HD + 1, :])
                        rinb = work.tile([1, QT], F32, tag="rin", bufs=4)
                        nc.vector.tensor_copy(rinb, ub[HD : HD + 1, :])
                        reca = work.tile([1, QT], F32, tag="rec", bufs=4)
                        nc.vector.reciprocal_approx_fast(reca, rina)
                        recb = work.tile([1, QT], F32, tag="rec", bufs=4)
                        nc.vector.reciprocal_approx_fast(recb, rinb)
                        rba = work.tile([64, QT], F32, tag="rb", bufs=4)
                        nc.gpsimd.partition_broadcast(rba, reca)
                        rbb = work.tile([64, QT], F32, tag="rb", bufs=4)
                        nc.gpsimd.partition_broadcast(rbb, recb)
                        ot = work.tile([128, QT], mmdt, tag="ot", bufs=6)
                        nc.vector.tensor_mul(ot[0:64, :], ua[0:HD, :], rba)
                        nc.vector.tensor_mul(ot[64:128, :], ub[0:HD, :], rbb)
                        if DEBUG_TAPS and bi == 0 and qi == 0 and pp == 0:
                            usb = work.tile([HD + 1, QT], F32, tag="dbgu")
                            nc.vector.tensor_copy(usb, ua)
                            nc.sync.dma_start(out=dbg_u[:, :], in_=usb)
                            nc.sync.dma_start(out=dbg_rb[:, :], in_=rba)
                            otf = work.tile([128, QT], F32, tag="dbgot")
                            nc.vector.tensor_copy(otf, ot)
                            nc.sync.dma_start(out=dbg_ot[:, :], in_=otf)
                        ots.append(ot)
                    queue_oproj(bi, q0, ots)
                if bi == Bv - 1:
                    # final tail: drain everything before the pool closes
                    drain(attnps, len(pending))
    nc.finalize()
    return nc


def host_inputs(x, wq, wk, wv, wo, Bv=B, Tv=T, mmdt=BF16):
    """Shard + pre-transpose inputs; returns list of 8 per-core input dicts."""
    import ml_dtypes

    np_mm = ml_dtypes.bfloat16 if mmdt is BF16 else np.float32
    cast = lambda a: np.ascontiguousarray(a).astype(np_mm)
    NTOK = Bv * Tv
    xT = cast(x.reshape(NTOK, H).T)

    # RoPE tables matching reference: emb = concat([freqs, freqs]) over dim,
    # rotate_half interleaved; sign folded into sin rows.
    inv_freq = (1.0 / (BASE ** (np.arange(0, HD, 2, dtype=np.float32) / np.float32(HD)))).astype(np.float32)
    t = np.arange(Tv, dtype=np.float32)
    freqs = np.outer(t, inv_freq)                       # [T, 32]
    emb = np.concatenate([freqs, freqs], axis=-1)       # [T, 64]
    cos = np.cos(emb).astype(np.float32)                # [T, 64]
    sin = np.sin(emb).astype(np.float32)
    sgn = np.where(np.arange(HD) % 2 == 0, -1.0, 1.0).astype(np.float32)
    sinS = sin * sgn[None, :]                           # sign-folded
    cosT2 = cast(np.vstack([cos.T, cos.T]))             # [128, T]
    sinT2 = cast(np.vstack([sinS.T, sinS.T]))           # [128, T]

    ki = np.arange(KT)[:, None]
    qi = np.arange(QT)[None, :]
    ok, bad = (0.0, MASKVAL) if MASK_ON_PE else (1.0, 0.0)
    masks = np.stack(
        [
            np.where(qi >= d * KT + ki, ok, bad).astype(np.float32)
            for d in range(NDIAG)
        ]
    )                                                    # [NDIAG, 128, QT]
    ident = cast(np.eye(128, dtype=np.float32))
    masks = cast(masks)

    in_maps = []
    for c in range(NCORES):
        qs = slice(c * QCH, (c + 1) * QCH)
        ks = slice(c * HD, (c + 1) * HD)
        wqT = cast(wq[qs].T)                             # [H, 256]
        wkvT = cast(np.concatenate([wk[ks].T, wv[ks].T], axis=1))  # [H, 128]
        woT = cast(wo[:, qs].T)                          # [256, H]
        in_maps.append(
            dict(xT=xT, wqT=wqT, wkvT=wkvT, woT=woT, cosT=cosT2, sinT=sinT2,
                 masks=masks, ident=ident)
        )
    return in_maps


_CACHED = {}


MMDT = BF16


def kernel(x, wq, wk, wv, wo):
    from concourse.bass_utils import run_bass_kernel_spmd

    if "nc" not in _CACHED:
        _CACHED["nc"] = build_nc(mmdt=MMDT)
    nc = _CACHED["nc"]
    in_maps = host_inputs(x, wq, wk, wv, wo, mmdt=MMDT)
    res = run_bass_kernel_spmd(nc, in_maps, core_ids=list(range(NCORES)))
    y = np.zeros((H, B * T), dtype=np.float64)
    for c in range(NCORES):
        y += res.results[c]["yT"].astype(np.float64)
    return np.ascontiguousarray(y.T.astype(np.float32).reshape(B, T, H))
